# revision 5
# baseline (speedup 1.0000x reference)
"""Trainium2 Bass kernel for nn_EntanglementPropagator (gnn_message_passing).

Math: the reference computes, for edges e=(src[e], dst[e]):
    eff_w[e,f]   = W[s,d,f] * cos(phase[s,d])
    signal[b,e,f]= x[b,s,f] * eff_w[e,f]
    out[b,n,f]   = (sum_{e: dst[e]==n} signal[b,e,f]) / max(out_deg[n],1)

Folding edge multiplicity M[s,d] (= # of (s,d) edges) and the 1/norm[d]
factor into a single per-(s,d) scale C[s,d] = cos(phase[s,d])*M[s,d]/norm[d]:

    out[b,d,f] = sum_s (W[s,d,f] * C[s,d]) * x[b,s,f]

i.e. F independent [B,N]x[N,DN] matmuls (contraction over source node s).

Sharding: dst-dimension split across the 8 cores (core c owns d in
[c*32,(c+1)*32)).  Each core reads W/8 + all of x (~17 MB) and writes out/8
(1 MB); no collectives needed.  The host only preprocesses the *integer*
edge tensors (multiplicity / degree counts); cos() and all heavy FP math run
on device (ScalarE Sin activation + DVE scaling + PE matmuls).
"""

import numpy as np

import concourse.bass as bass
import concourse.mybir as mybir
import concourse.tile as tile
from concourse import bacc
from concourse.bass_utils import run_bass_kernel_spmd

N = 256          # nodes
F = 256          # feature dim
B = 32           # batch
N_CORES = 8
DN = N // N_CORES        # dst nodes per core = 32
KB = 2                   # source-node partition blocks (s: 2 x 128)
FH = 128                 # f-range per DMA piece (512B contiguous runs)
FC = 64                  # f-range per PSUM chunk (4 banks)
F32 = mybir.dt.float32

HALF_PI = float(np.pi / 2.0)


def build_body(tc, w, xs, ph, ms, out):
    """Emit one iteration of the kernel body.

    w   [N, DN, F]  DRAM  - this core's W[:, d0:d0+DN, :]
    xs  [B, N, F]   DRAM  - full node_features
    ph  [N, DN]     DRAM  - this core's phase[:, d0:d0+DN]
    ms  [N, DN]     DRAM  - multiplicity/norm scale (host, from int tensors)
    out [B, DN, F]  DRAM  - this core's output slice
    """
    nc = tc.nc

    with (
        tc.tile_pool(name="cpool", bufs=2) as cpool,
        tc.tile_pool(name="wpool", bufs=4) as wpool,
        tc.tile_pool(name="xpool", bufs=4) as xpool,
        tc.tile_pool(name="opool", bufs=1) as opool,
        tc.tile_pool(name="ppool", bufs=2, space="PSUM") as ppool,
    ):
        # --- per-(s,d) scale C = cos(phase) * M/norm, layout [s_part, d].
        # The Sin LUT is only accurate on ~[-pi, pi], so use the half-angle
        # form cos(x) = 2*sin^2(x/2 - pi/2) - 1 (argument stays in
        # [-pi/2, pi/2] for x in [0, 2pi]).
        bias_t = cpool.tile([128, 1], F32, tag="bias")
        nc.vector.memset(bias_t, -HALF_PI)
        c_t = {}
        for kb in range(KB):
            sl = slice(kb * 128, (kb + 1) * 128)
            ph_t = cpool.tile([128, DN], F32, tag="ph")
            nc.sync.dma_start(out=ph_t, in_=ph[sl, :])
            ms_t = cpool.tile([128, DN], F32, tag="ms")
            nc.sync.dma_start(out=ms_t, in_=ms[sl, :])
            c = cpool.tile([128, DN], F32, tag="c")
            nc.scalar.activation(out=c, in_=ph_t,
                                 func=mybir.ActivationFunctionType.Sin,
                                 bias=bias_t, scale=0.5)
            nc.vector.tensor_mul(out=c, in0=c, in1=c)
            nc.vector.tensor_scalar(out=c, in0=c, scalar1=2.0, scalar2=-1.0,
                                    op0=mybir.AluOpType.mult,
                                    op1=mybir.AluOpType.add)
            nc.vector.tensor_mul(out=c, in0=c, in1=ms_t)
            c_t[kb] = c

        # --- big DMA pieces: W' = W * C (scaled in SBUF), X transposed to
        #     [s_part, b, f].  fh-major order so f-chunk matmuls start early.
        w_t = {}
        x_t = {}
        for fh in range(F // FH):
            fsl = slice(fh * FH, (fh + 1) * FH)
            for kb in range(KB):
                ssl = slice(kb * 128, (kb + 1) * 128)
                wt = wpool.tile([128, DN, FH], F32, tag="w")
                nc.sync.dma_start(out=wt, in_=w[ssl, :, fsl])
                nc.vector.tensor_mul(
                    out=wt, in0=wt,
                    in1=c_t[kb][:, :, None].broadcast_to([128, DN, FH]))
                w_t[kb, fh] = wt

                xt = xpool.tile([128, B, FH], F32, tag="x")
                nc.sync.dma_start(
                    out=xt, in_=xs[:, ssl, fsl].rearrange("b s f -> s b f"))
                x_t[kb, fh] = xt

        # --- matmuls: per f, out[b, d] += X_f[s, b].T @ W'_f[s, d]
        out_sb = opool.tile([B, DN, F], F32)
        n_chunks = F // FC
        for fc in range(n_chunks):
            fh = (fc * FC) // FH
            ps = ppool.tile([B, FC, DN], F32)
            # NOTE: an accumulation group (start..stop) on a PSUM region must
            # be contiguous on PE - interleaving other matmuls between a
            # start and its stop corrupts the accumulation.  So kb is the
            # inner loop.
            for fl in range(FC):
                f_local = fc * FC + fl - fh * FH
                for kb in range(KB):
                    nc.tensor.matmul(
                        ps[:, fl, :],
                        lhsT=x_t[kb, fh][:, :, f_local],
                        rhs=w_t[kb, fh][:, :, f_local],
                        start=(kb == 0),
                        stop=(kb == KB - 1),
                    )
            # transpose (f,d) -> (d,f) while draining PSUM
            nc.vector.tensor_copy(
                out=out_sb[:, :, fc * FC:(fc + 1) * FC],
                in_=ps.rearrange("p f d -> p d f"))

        nc.sync.dma_start(out=out, in_=out_sb)


def build_program(n_repeat=1):
    nc = bacc.Bacc("TRN2", target_bir_lowering=False, debug=False,
                   num_devices=N_CORES)
    w = nc.dram_tensor("w", [N, DN, F], F32, kind="ExternalInput").ap()
    xs = nc.dram_tensor("xs", [B, N, F], F32, kind="ExternalInput").ap()
    ph = nc.dram_tensor("ph", [N, DN], F32, kind="ExternalInput").ap()
    ms = nc.dram_tensor("ms", [N, DN], F32, kind="ExternalInput").ap()
    out = nc.dram_tensor("out", [B, DN, F], F32, kind="ExternalOutput").ap()

    with tile.TileContext(nc) as tc:
        for _ in range(n_repeat):
            build_body(tc, w, xs, ph, ms, out)
    nc.compile()
    return nc


def host_prep(phase, src, dst):
    """Per-(s,d) multiplicity / out-degree normalization from the integer
    edge tensors.  Returns ms [N, N] float32 with ms[s,d] = M[s,d]/norm[d]."""
    src = np.asarray(src).astype(np.int64)
    dst = np.asarray(dst).astype(np.int64)
    counts = np.bincount(src, minlength=N).astype(np.float64)
    norm = np.maximum(counts, 1.0)                      # per-node out-degree
    mult = np.bincount(src * N + dst, minlength=N * N).astype(np.float64)
    mult = mult.reshape(N, N)
    ms = (mult / norm[None, :]).astype(np.float32)
    return ms


_PROGRAM_CACHE = {}


def get_program(n_repeat=1):
    if n_repeat not in _PROGRAM_CACHE:
        _PROGRAM_CACHE[n_repeat] = build_program(n_repeat)
    return _PROGRAM_CACHE[n_repeat]


def make_in_maps(node_features, W, phase, src, dst):
    node_features = np.ascontiguousarray(np.asarray(node_features, dtype=np.float32))
    W = np.asarray(W, dtype=np.float32)
    phase = np.asarray(phase, dtype=np.float32)
    ms = host_prep(phase, src, dst)
    in_maps = []
    for c in range(N_CORES):
        dsl = slice(c * DN, (c + 1) * DN)
        in_maps.append({
            "w": np.ascontiguousarray(W[:, dsl, :]),
            "xs": node_features,
            "ph": np.ascontiguousarray(phase[:, dsl]),
            "ms": np.ascontiguousarray(ms[:, dsl]),
        })
    return in_maps


def kernel(node_features, W, phase, src, dst):
    nc = get_program(1)
    in_maps = make_in_maps(node_features, W, phase, src, dst)
    res = run_bass_kernel_spmd(nc, in_maps, list(range(N_CORES)))
    return np.concatenate([res.results[c]["out"] for c in range(N_CORES)],
                          axis=1)


# revision 24
# speedup vs baseline: 5.3398x; 5.3398x over previous
"""Trainium2 Bass kernel for nn_EntanglementPropagator (gnn_message_passing).

Math: the reference computes, for edges e=(src[e], dst[e]):
    eff_w[e,f]   = W[s,d,f] * cos(phase[s,d])
    signal[b,e,f]= x[b,s,f] * eff_w[e,f]
    out[b,n,f]   = (sum_{e: dst[e]==n} signal[b,e,f]) / max(out_deg[n],1)

Folding edge multiplicity M[s,d] (= # of (s,d) edges) and the 1/norm[d]
factor into a single per-(s,d) scale C[s,d] = cos(phase[s,d])*M[s,d]/norm[d]:

    out[b,d,f] = sum_s (W[s,d,f] * C[s,d]) * x[b,s,f]

i.e. F independent [B,N]x[N,DN] matmuls (contraction over source node s).

Sharding: dst-dimension split across the 8 cores (core c owns d in
[c*32,(c+1)*32)).  Each core reads W/8 + all of x (~17 MB) and writes out/8
(1 MB); no collectives needed.  The host only does layout work (slice /
transpose) plus preprocessing of the *integer* edge tensors (multiplicity /
degree counts); cos() and all heavy FP math run on device.

Key HW findings baked into the design:
  * fp32 matmuls self-load weights (no LDWEIGHTS pull-ahead) at 4 cycles
    per column, so many small matmuls are issue-bound (~168ns for 32x32).
    Packing 4 f-planes per matmul (M=N=128, ignoring the off-diagonal
    f-cross blocks) measures 318ns/matmul -> 2.1x less PE time total.
  * A packed operand must merge to a SINGLE free dim (walrus restriction),
    hence both W and X are kept f-major on SBUF ([s, f, d] / [s, f, b]),
    which also makes every DMA piece fully contiguous per partition.
  * PSUM accumulation groups must be contiguous on PE, so the two
    source-halves (kb) accumulate via SBUF: kb0 drains with a copy (ACT),
    kb1 with an add (DVE).
  * A matmul output must not cross a PSUM bank boundary.
"""

import numpy as np

import concourse.mybir as mybir
import concourse.tile as tile
from concourse import bacc
from concourse.bass_utils import run_bass_kernel_spmd

N = 256          # nodes
F = 256          # feature dim
B = 32           # batch
N_CORES = 8
DN = N // N_CORES        # dst nodes per core = 32
KB = 2                   # source-node partition blocks (s: 2 x 128)
FC = 32                  # f-range per PSUM chunk ([128, 8, 128] = 2 banks)
FP = 4                   # f-planes packed per matmul (M = FP*DN, N = FP*B)
F32 = mybir.dt.float32

HALF_PI = float(np.pi / 2.0)


def build_body(tc, w, xs, phm, out):
    """Emit one iteration of the kernel body.

    w   [N, F, DN]  DRAM  - W[:, d0:d0+DN, :] transposed to f-major
    xs  [N, F, B]   DRAM  - node_features transposed to [node, feat, batch]
    phm [2, N, DN]  DRAM  - phase[:, dsl] and M/norm scale (from int tensors)
    out [B, DN, F]  DRAM  - this core's output slice
    """
    nc = tc.nc

    with (
        tc.tile_pool(name="cpool", bufs=2) as cpool,
        tc.tile_pool(name="wpool", bufs=4) as wpool,
        tc.tile_pool(name="xpool", bufs=4) as xpool,
        tc.tile_pool(name="opool", bufs=1) as opool,
        tc.tile_pool(name="ppool", bufs=4, space="PSUM") as ppool,
    ):
        # --- per-(s,d) scale C = cos(phase) * M/norm, layout [s_part, d].
        # The Sin LUT is only accurate on ~[-pi, pi], so use the half-angle
        # form cos(x) = 2*sin^2(x/2 - pi/2) - 1 (argument stays in
        # [-pi/2, pi/2] for x in [0, 2pi]).
        bias_t = cpool.tile([128, 1], F32, tag="bias")
        nc.vector.memset(bias_t, -HALF_PI)
        phm_t = cpool.tile([128, 2, KB, DN], F32, tag="phm")
        nc.sync.dma_start(
            out=phm_t, in_=phm.rearrange("t (k p) d -> p t k d", k=KB))
        c_t = {}
        for kb in range(KB):
            c = cpool.tile([128, DN], F32, tag="c")
            nc.scalar.activation(out=c, in_=phm_t[:, 0, kb, :],
                                 func=mybir.ActivationFunctionType.Sin,
                                 bias=bias_t, scale=0.5)
            nc.vector.tensor_mul(out=c, in0=c, in1=c)
            nc.vector.tensor_scalar(out=c, in0=c, scalar1=2.0, scalar2=-1.0,
                                    op0=mybir.AluOpType.mult,
                                    op1=mybir.AluOpType.add)
            nc.vector.tensor_mul(out=c, in0=c, in1=phm_t[:, 1, kb, :])
            c_t[kb] = c

        # out_sb layout [d, b, f]: the packed matmul puts (f-plane, d) on
        # PSUM partitions, so drains land d-major; the out DMA restores the
        # [b, d, f] HBM order (partition stride = d stride).
        out_sb = opool.tile([DN, B, F], F32)

        # --- stream pieces and compute.  A piece is (kb, f0, f1): one W DMA
        # + scale + one X DMA + packed matmuls + PSUM drains.  All pieces
        # are fully contiguous per partition (f-major layouts), so piece
        # granularity is free - the tail pieces are small so that little
        # work remains after the last input byte lands.
        out_groups = [
            # (f-range of the out DMA, pieces)
            (slice(0, 128), [(0, 0, 128), (1, 0, 128)]),
            (slice(128, 256), [(0, 128, 256), (1, 128, 224), (1, 224, 256)]),
        ]
        for osl_f, pieces in out_groups:
            for kb, f0, f1 in pieces:
                fsl = slice(f0, f1)
                fw = f1 - f0
                ssl = slice(kb * 128, (kb + 1) * 128)
                wt = wpool.tile([128, 128, DN], F32, tag="w")
                wt = wt[:, :fw, :]
                nc.sync.dma_start(out=wt, in_=w[ssl, fsl, :])
                # W' = W * C  (broadcast C over f) on DVE
                nc.vector.tensor_mul(
                    out=wt, in0=wt,
                    in1=c_t[kb][:, None, :].broadcast_to([128, fw, DN]))

                xt = xpool.tile([128, 128, B], F32, tag="x")
                xt = xt[:, :fw, :]
                nc.sync.dma_start(out=xt, in_=xs[ssl, fsl, :])

                for ci in range(fw // FC):
                    # psum [(fj,d) = 128, g, (fi,b) = 128]; each matmul
                    # writes 512B/partition contiguous (bank-contained).
                    ps = ppool.tile([FP * DN, FC // FP, FP * B], F32)
                    for g in range(FC // FP):
                        fg = ci * FC + g * FP
                        nc.tensor.matmul(
                            ps[:, g],
                            lhsT=wt[:, fg:fg + FP, :].rearrange(
                                "s f d -> s (f d)"),
                            rhs=xt[:, fg:fg + FP, :].rearrange(
                                "s f b -> s (f b)"),
                            start=True, stop=True)
                    # drain diagonal (fi == fj) blocks; f = base + g*FP + fi
                    base = f0 + ci * FC
                    for fi in range(FP):
                        src = ps[fi * DN:(fi + 1) * DN, :,
                                 fi * B:(fi + 1) * B]
                        dst = out_sb[:, :, base + fi:base + FC:FP] \
                            .rearrange("d b g -> d g b")
                        if kb == 0:
                            # PSUM -> SBUF drain on ACT (keeps DVE free
                            # for the W-scaling muls)
                            nc.scalar.copy(out=dst, in_=src)
                        else:
                            nc.vector.tensor_add(out=dst, in0=dst, in1=src)
            # drain this group's f-range of the output.  Issued on the ACT
            # HWDGE queue: its sem wait (adds done) must not stall the
            # input stream on the sync queue.
            nc.scalar.dma_start(
                out=out[:, :, osl_f].rearrange("b d f -> d b f"),
                in_=out_sb[:, :, osl_f])


def build_program(n_repeat=1, loop_k=None):
    nc = bacc.Bacc("TRN2", target_bir_lowering=False, debug=False,
                   num_devices=N_CORES)
    w = nc.dram_tensor("w", [N, F, DN], F32, kind="ExternalInput").ap()
    xs = nc.dram_tensor("xs", [N, F, B], F32, kind="ExternalInput").ap()
    phm = nc.dram_tensor("phm", [2, N, DN], F32, kind="ExternalInput").ap()
    out = nc.dram_tensor("out", [B, DN, F], F32, kind="ExternalOutput").ap()

    with tile.TileContext(nc) as tc:
        if loop_k is not None:
            # HW loop around the body - for wall-clock timing with enough
            # iterations to swamp the host<->device dispatch noise.
            with tc.For_i(0, loop_k, 1):
                for _ in range(n_repeat):
                    build_body(tc, w, xs, phm, out)
        else:
            for _ in range(n_repeat):
                build_body(tc, w, xs, phm, out)
    nc.compile()
    return nc


def host_prep(phase, src, dst):
    """Per-(s,d) multiplicity / out-degree normalization from the integer
    edge tensors.  Returns ms [N, N] float32 with ms[s,d] = M[s,d]/norm[d]."""
    src = np.asarray(src).astype(np.int64)
    dst = np.asarray(dst).astype(np.int64)
    counts = np.bincount(src, minlength=N).astype(np.float64)
    norm = np.maximum(counts, 1.0)                      # per-node out-degree
    mult = np.bincount(src * N + dst, minlength=N * N).astype(np.float64)
    mult = mult.reshape(N, N)
    ms = (mult / norm[None, :]).astype(np.float32)
    return ms


_PROGRAM_CACHE = {}


def get_program(n_repeat=1, loop_k=None):
    key = (n_repeat, loop_k)
    if key not in _PROGRAM_CACHE:
        _PROGRAM_CACHE[key] = build_program(n_repeat, loop_k)
    return _PROGRAM_CACHE[key]


def make_in_maps(node_features, W, phase, src, dst):
    node_features = np.asarray(node_features, dtype=np.float32)
    W = np.asarray(W, dtype=np.float32)
    phase = np.asarray(phase, dtype=np.float32)
    ms = host_prep(phase, src, dst)
    # f-major layouts (see module docstring): pure transposes, no math.
    xT = np.ascontiguousarray(node_features.transpose(1, 2, 0))  # [N, F, B]
    in_maps = []
    for c in range(N_CORES):
        dsl = slice(c * DN, (c + 1) * DN)
        in_maps.append({
            "w": np.ascontiguousarray(W[:, dsl, :].transpose(0, 2, 1)),
            "xs": xT,
            "phm": np.ascontiguousarray(
                np.stack([phase[:, dsl], ms[:, dsl]], axis=0)),
        })
    return in_maps


def kernel(node_features, W, phase, src, dst):
    nc = get_program(1)
    in_maps = make_in_maps(node_features, W, phase, src, dst)
    res = run_bass_kernel_spmd(nc, in_maps, list(range(N_CORES)))
    return np.concatenate([res.results[c]["out"] for c in range(N_CORES)],
                          axis=1)
